# revision 1
# baseline (speedup 1.0000x reference)
"""AugLUT Trainium2 kernel: per-batch 20-knot LUT applied to x via
piecewise-linear interpolation, multi-engine knot-split (372us/core measured,
vs 516us for the previous all-DVE/ACT+PE hybrid).

f(t) = y0 + sum_{k=0..18} D'_k * clamp(t-k, 0, 1),  t = 19x, D'_k = y_{k+1}-y_k.

Per [128, 2048] chunk the 19 slope-knots are split across engines (measured
per-op costs in ns for [128x2048]):
- PAIRS (8 knots, k=0..7): custom-DVE clamp-pair chain, 4 ops x 2311ns,
  self-accumulating; seeded with the constant K via a once-built broadcast
  tile so no separate bias stage is needed. fp16 accumulator.
- TS (6 knots): DVE stock tensor_scalar clamp(t,k,k+1) in fp16 at 624ns --
  the 4x_2p DVE perf mode fires (all-SBUF, 2-byte dtype, packed); custom DVE
  ops can never use perf modes (8 ALU slices; 2x needs <=3), which is why
  the stock-op basis beats more custom pairs on DVE time.
- RELU (5 knots, top block k=14..18): ACT Relu(19x-k) -> fp16 at 1903ns,
  with telescoped coefficients so a contiguous top block costs exactly b ops.
- TS/RELU bases + pair partial are accumulated on PE via diagonal fp16
  matmuls into PSUM (1066ns/knot incl ldweights, 512-col subtiles; wider
  matmuls are ISA-invalid, fp32 would be 4-pass). GPSIMD is useless here
  (~21us/op software loops). DVE->PSUM prewrite + start=False accumulation
  gives wrong results on HW; the identity inject matmul is required.

ACT also produces t16 = fp16(19x) (1 op) and the final PSUM->SBUF copy.
Engine busy per chunk ~ DVE 13.0us, ACT 13.3us, PE 12.8us vs DMA floor
6.5us (324 GB/s/core measured) -> compute-bound at ~13.8us/chunk.

Sharding: pure data parallel -- batch b -> NeuronCore b (8 cores); the tiny
LUT/weight tensors ride along as per-core inputs. rel_err ~ 7.8e-3 (fp16
bases/accumulator; gate is 2e-2).
"""

import sys

if "/opt/trn_rl_repo" not in sys.path:
    sys.path.insert(0, "/opt/trn_rl_repo")

import numpy as np

import concourse.bacc as bacc
import concourse.dve_ops as dve_ops_mod
import concourse.mybir as mybir
from concourse import bass_utils
from concourse.dve_ops import DveOp
from concourse.dve_spec import (
    C0,
    C1,
    C2,
    Latch,
    One,
    Spec,
    Src0,
    Src1,
    Zero,
    lower,
    maxx,
    minn,
    _has_src1,
)
from concourse.dve_uop import DveOpSpec
from concourse.tile import TileContext

N_BINS = 20
EPS = 1e-5
BATCH = 8
SPATIAL = (192, 192, 192)
N_ELEM = 192 * 192 * 192  # 7_077_888
P = 128
FREE = N_ELEM // P  # 55296
CHUNK = 2048
PSUM_SUB = 512


# --------------------------------------------------------------------------
# Custom DVE pair ops (same as baseline kernel)
# --------------------------------------------------------------------------
def _pair_body(with_acc: bool):
    e = Src0 - C2
    r = maxx(e, Zero)
    c1 = minn(r, One)
    p1 = c1 * C0
    c2 = minn(r, One + One)
    if with_acc:
        a = Src1 + p1
        p2 = c2 * Latch(maxx(C1, C1))
        return a + p2
    p2 = c2 * C1
    return p1 + p2


def _np_pair(in0, in1, s0, s1, imm2, with_acc):
    e = in0.astype(np.float32) - np.float32(imm2)
    c1 = np.minimum(np.maximum(e, np.float32(0)), np.float32(1))
    c2 = np.minimum(np.maximum(e, np.float32(0)), np.float32(2))
    s0 = np.asarray(s0, dtype=np.float32)
    s1 = np.asarray(s1, dtype=np.float32)
    r = c1 * s0 + c2 * s1
    if with_acc:
        r = r + in1
    return r.astype(np.float32)


def _register(name: str, spec: Spec) -> DveOp:
    for op in dve_ops_mod.OPS:
        if op.name == name:
            return op
    row = dve_ops_mod._CUSTOM_DVE_ROW_BASE + len(dve_ops_mod.OPS)
    assert row < 0x20, "custom-DVE row overflow"
    sha = {}
    for ver in ("v3", "v4"):
        try:
            s = DveOpSpec(
                name=name,
                opcode=row,
                uops=lower(spec, ver=ver),
                rd1_en=_has_src1(spec),
            )
            sha[ver] = s.sha(ver)
        except Exception:
            pass
    op = DveOp(name, spec, subdim=False, uops_sha=sha)
    dve_ops_mod.OPS.append(op)
    dve_ops_mod.CUSTOM_DVE_SPECS[name] = spec
    dve_ops_mod._SUB_OPCODE_FOR_NAME[name] = row
    return op


AUGLUT_PAIR = _register(
    "AUGLUT_PAIR",
    Spec(
        body=_pair_body(with_acc=True),
        reference=lambda in0, in1, s0, s1, imm2: _np_pair(in0, in1, s0, s1, imm2, True),
    ),
)

AUGLUT_PAIR_INIT = _register(
    "AUGLUT_PAIR_INIT",
    Spec(
        body=_pair_body(with_acc=False),
        reference=lambda in0, in1, s0, s1, imm2: _np_pair(
            in0, None, s0, s1, imm2, False
        ),
    ),
)


# --------------------------------------------------------------------------
# Knot split
# --------------------------------------------------------------------------
def knot_split(p: int, a: int, b: int, c: int):
    """19 slope-knots -> (pair knots 0..2p-1, ts knots, relu top block, pool).
    relu block is the contiguous top [19-b .. 18]; ts/pool fill the middle."""
    assert 2 * p + a + b + c == 19, (p, a, b, c)
    pair_knots = list(range(0, 2 * p))
    relu_knots = list(range(19 - b, 19))
    mid = [k for k in range(2 * p, 19 - b)]
    assert len(mid) == a + c
    ts_knots = mid[:a]
    pool_knots = mid[a:]
    return pair_knots, ts_knots, relu_knots, pool_knots


# --------------------------------------------------------------------------
# Bass module
# --------------------------------------------------------------------------
def build_module(
    reps: int = 1,
    chunk: int = CHUNK,
    p: int = 4,
    a: int = 6,
    b: int = 5,
    c: int = 0,
    bufs: int = 3,
    basis_bufs: int = 6,
    t_f32_pairs: bool = False,
    psum_sub: int = PSUM_SUB,
    copy_eng: str = "act",
    dve_copies: int = 0,  # chunks per 27 whose PSUM->SBUF copy runs on DVE
    prewrite_psum: bool = False,  # pair chain writes PSUM; else inject matmul
    pipelined_copy: bool = False,  # emit chunk j's PSUM->SBUF copy after chunk j+1's compute
    ts_first: bool = False,  # emit ts-clamp knots before the pair chain on DVE
):
    nc = bacc.Bacc("TRN2", target_bir_lowering=False, debug=False, num_devices=BATCH)

    f32 = mybir.dt.float32
    f16 = mybir.dt.float16
    pair_knots, ts_knots, relu_knots, pool_knots = knot_split(p, a, b, c)
    assert p > 0, "pair chain carries the K constant; p >= 1 required"
    n_mat = a + b + c + (0 if prewrite_psum else 1)
    n_sub = chunk // psum_sub

    x_d = nc.dram_tensor("x", [P, FREE], f32, kind="ExternalInput")
    # lut[:, 0:2*p]: pair-op scalars (s0,s1 per pair);
    lut_d = nc.dram_tensor("lut", [P, N_BINS], f32, kind="ExternalInput")
    # wts: n_mat diag fp16 weight matrices [128, n_mat*128]
    wts_d = nc.dram_tensor("wts", [P, max(1, n_mat) * P], f16, kind="ExternalInput")
    # ab[:, 0]: final bias K; ab[:, 1:1+b]: -k biases for the relu knots
    ab_d = nc.dram_tensor("ab", [P, 8], f32, kind="ExternalInput")
    o_d = nc.dram_tensor("o", [P, FREE], f32, kind="ExternalOutput")

    x_ap = x_d.ap()
    o_ap = o_d.ap()
    n_chunks = FREE // chunk
    assert n_chunks * chunk == FREE, (chunk, FREE)

    with TileContext(nc) as tc:
        with (
            tc.tile_pool(name="lutp", bufs=1) as lutp,
            tc.tile_pool(name="xp", bufs=bufs) as xp,
            tc.tile_pool(name="tp", bufs=bufs) as tp,
            tc.tile_pool(name="bp", bufs=basis_bufs) as bp,
            tc.tile_pool(name="op", bufs=bufs) as op_,
            tc.tile_pool(name="pairp", bufs=max(2, bufs - 1)) as pairp,
            tc.tile_pool(name="psum", bufs=2, space="PSUM") as pp,
        ):
            lut_t = lutp.tile([P, N_BINS], f32)
            nc.sync.dma_start(out=lut_t[:], in_=lut_d.ap()[:])
            wts_t = lutp.tile([P, max(1, n_mat) * P], f16)
            nc.sync.dma_start(out=wts_t[:], in_=wts_d.ap()[:])
            ab_t = lutp.tile([P, 8], f32)
            nc.sync.dma_start(out=ab_t[:], in_=ab_d.ap()[:])
            # K broadcast tile (built once): kt = 0*z + K  (not in-place)
            zt = lutp.tile([P, chunk], f32)
            nc.vector.memset(zt[:], 0.0)
            kt = lutp.tile([P, chunk], f32)
            nc.scalar.activation(
                out=kt[:],
                in_=zt[:],
                func=mybir.ActivationFunctionType.Identity,
                bias=ab_t[:, 0:1],
                scale=1.0,
            )

            dve_copy_set = (
                {int(round((i + 0.5) * n_chunks / dve_copies)) for i in range(dve_copies)}
                if dve_copies > 0
                else set()
            )

            def chunk_body(j):
                sl = slice(j * chunk, (j + 1) * chunk)
                xt = xp.tile([P, chunk], f32, tag="x")
                nc.sync.dma_start(out=xt[:], in_=x_ap[:, sl])

                # t16 = fp16(19 x)  (Copy func: float bias, imm scale)
                t16 = tp.tile([P, chunk], f16, tag="t16")
                nc.scalar.mul(out=t16[:], in_=xt[:], mul=19.0)
                if t_f32_pairs and p > 0:
                    tt = tp.tile([P, chunk], f32, tag="t32")
                    nc.scalar.mul(out=tt[:], in_=xt[:], mul=19.0)
                else:
                    tt = t16

                ps = pp.tile([P, chunk], f32)
                mi = 0  # matmul index / weight slot

                def accumulate(basis_tile, mi):
                    # prewrite_psum: psum pre-written by the DVE pair chain -> never start
                    w_sl = wts_t[:, mi * P : (mi + 1) * P]
                    for i in range(n_sub):
                        ss = slice(i * psum_sub, (i + 1) * psum_sub)
                        nc.tensor.matmul(
                            ps[:, ss],
                            w_sl,
                            basis_tile[:, ss],
                            start=(False if prewrite_psum else mi == 0),
                            stop=(mi == n_mat - 1),
                        )

                def emit_pairs():
                    acc = pairp.tile([P, chunk], f16, tag="acc")
                    accf = pairp.tile([P, chunk], f16, tag="accf")
                    prev = kt
                    for pr in range(p):
                        last = pr == p - 1
                        out_t = ps if (last and prewrite_psum) else (accf if last else acc)
                        nc.vector._custom_dve(
                            AUGLUT_PAIR,
                            out=out_t[:],
                            in0=tt[:],
                            in1=prev[:],
                            s0=lut_t[:, 2 * pr : 2 * pr + 1],
                            s1=lut_t[:, 2 * pr + 1 : 2 * pr + 2],
                            imm2=float(2 * pr),
                        )
                        prev = acc
                    return accf

                if not ts_first:
                    accf = emit_pairs()

                # --- DVE ts-clamp knots ---
                for k in ts_knots:
                    bt = bp.tile([P, chunk], f16, tag="basis")
                    nc.vector.tensor_scalar(
                        out=bt[:],
                        in0=t16[:],
                        scalar1=float(k),
                        scalar2=float(k + 1),
                        op0=mybir.AluOpType.max,
                        op1=mybir.AluOpType.min,
                    )
                    accumulate(bt, mi)
                    mi += 1

                # --- ACT relu knots (top block) ---
                for idx, k in enumerate(relu_knots):
                    rt = bp.tile([P, chunk], f16, tag="basis")
                    nc.scalar.activation(
                        out=rt[:],
                        in_=xt[:],
                        func=mybir.ActivationFunctionType.Relu,
                        bias=ab_t[:, 1 + idx : 2 + idx],
                        scale=19.0,
                    )
                    accumulate(rt, mi)
                    mi += 1

                # --- Pool ts-clamp knots ---
                for k in pool_knots:
                    bt = bp.tile([P, chunk], f16, tag="basis")
                    nc.gpsimd.tensor_scalar(
                        out=bt[:],
                        in0=t16[:],
                        scalar1=float(k),
                        scalar2=float(k + 1),
                        op0=mybir.AluOpType.max,
                        op1=mybir.AluOpType.min,
                    )
                    accumulate(bt, mi)
                    mi += 1

                if ts_first:
                    accf = emit_pairs()
                if not prewrite_psum:
                    accumulate(accf, mi)
                    mi += 1
                assert mi == n_mat, (mi, n_mat)

                return ps, sl, j

            def emit_copy(pending):
                ps, sl, j = pending
                os_t = op_.tile([P, chunk], f32, tag="os")
                if copy_eng == "act" and j not in dve_copy_set:
                    nc.scalar.copy(out=os_t[:], in_=ps[:])
                else:
                    nc.vector.tensor_copy(out=os_t[:], in_=ps[:])
                nc.sync.dma_start(out=o_ap[:, sl], in_=os_t[:])

            def body():
                if not pipelined_copy:
                    for j in range(n_chunks):
                        emit_copy(chunk_body(j))
                    return
                prev = None
                for j in range(n_chunks):
                    cur = chunk_body(j)
                    if prev is not None:
                        emit_copy(prev)
                    prev = cur
                emit_copy(prev)

            if reps == 1:
                body()
            else:
                with tc.For_i(
                    0,
                    reps,
                    1,
                    hint_engines=(
                        mybir.EngineType.DVE,
                        mybir.EngineType.SP,
                        mybir.EngineType.Activation,
                        mybir.EngineType.PE,
                        mybir.EngineType.Pool,
                    ),
                ):
                    body()

    nc.finalize()
    return nc


_MODULE_CACHE: dict[tuple, object] = {}


def _get_module(reps: int = 1, **cfg):
    key = (reps, tuple(sorted(cfg.items())))
    if key not in _MODULE_CACHE:
        _MODULE_CACHE[key] = build_module(reps, **cfg)
    return _MODULE_CACHE[key]


# --------------------------------------------------------------------------
# Host-side LUT prep
# --------------------------------------------------------------------------
def _make_luts(ran_y: np.ndarray, p: int, a: int, b: int, c: int, inject: bool = False):
    """ran_y [8, 20] -> (lut [8,128,20] f32, wts [8,128,n_mat*128] f16,
    ab [8,128,4] f32)."""
    y = ran_y.astype(np.float32)
    ymin = y.min(axis=1, keepdims=True)
    ymax = y.max(axis=1, keepdims=True)
    y = (y - ymin) / (ymax - ymin + np.float32(EPS))

    D = y[:, 1:] - y[:, :-1]  # [8, 19] slope deltas D'_k
    y0 = y[:, 0]  # [8]

    pair_knots, ts_knots, relu_knots, pool_knots = knot_split(p, a, b, c)
    n_mat = a + b + c + (1 if inject else 0)

    # pair scalars: for pair j (knots 2j, 2j+1): s0 = D_2j - D_{2j+1}, s1 = D_{2j+1}
    lut = np.zeros((BATCH, N_BINS), np.float32)
    for j in range(p):
        lut[:, 2 * j] = D[:, 2 * j] - D[:, 2 * j + 1]
        lut[:, 2 * j + 1] = D[:, 2 * j + 1]
    lut_full = np.broadcast_to(lut[:, None, :], (BATCH, P, N_BINS)).copy()

    # PE diag weights, one [128,128] diag slot per matmul, fp16
    wts = np.zeros((BATCH, P, max(1, n_mat) * P), np.float16)
    di = np.arange(P)
    K = y0.copy()  # final-bias constant
    mi = 0
    for k in ts_knots:
        wts[:, di, mi * P + di] = D[:, k][:, None].astype(np.float16)
        K = K - D[:, k] * np.float32(k)
        mi += 1
    # telescoped relu coefficients on the top block
    for idx, k in enumerate(relu_knots):
        if idx == 0:
            coef = D[:, k]
        else:
            coef = D[:, k] - D[:, k - 1]
        wts[:, di, mi * P + di] = coef[:, None].astype(np.float16)
        mi += 1
    for k in pool_knots:
        wts[:, di, mi * P + di] = D[:, k][:, None].astype(np.float16)
        K = K - D[:, k] * np.float32(k)
        mi += 1
    if inject:
        wts[:, di, mi * P + di] = np.float16(1.0)
        mi += 1
    assert mi == n_mat, (mi, n_mat)

    ab = np.zeros((BATCH, P, 8), np.float32)
    ab[:, :, 0] = K[:, None]
    for idx, k in enumerate(relu_knots):
        ab[:, :, 1 + idx] = -np.float32(k)
    return lut_full, wts, ab


# --------------------------------------------------------------------------
# Entry point
# --------------------------------------------------------------------------
CFG = dict(p=4, a=6, b=5, c=0, bufs=3, basis_bufs=6, prewrite_psum=False)


def kernel(x: np.ndarray, ran_y: np.ndarray, _reps: int = 1, **_cfg) -> np.ndarray:
    x = np.asarray(x, dtype=np.float32)
    ran_y = np.asarray(ran_y, dtype=np.float32)
    assert x.shape == (BATCH, *SPATIAL), x.shape
    assert ran_y.shape == (BATCH, N_BINS), ran_y.shape

    cfg = {**CFG, **_cfg}
    nc = _get_module(_reps, **cfg)
    lut, wts, ab = _make_luts(
        ran_y,
        cfg["p"],
        cfg["a"],
        cfg["b"],
        cfg["c"],
        inject=not cfg.get("prewrite_psum", True),
    )
    xr = np.ascontiguousarray(x.reshape(BATCH, P, FREE))
    in_maps = []
    for bi in range(BATCH):
        in_maps.append({"x": xr[bi], "lut": lut[bi], "wts": wts[bi], "ab": ab[bi]})

    res = bass_utils.run_bass_kernel_spmd(nc, in_maps, core_ids=list(range(BATCH)))
    out = np.stack([res.results[bi]["o"] for bi in range(BATCH)], axis=0)
    return out.reshape(BATCH, *SPATIAL)



# revision 2
# speedup vs baseline: 1.1402x; 1.1402x over previous
"""AugLUT Trainium2 kernel: per-batch 20-knot LUT applied to x via
piecewise-linear interpolation, multi-engine knot-split.

f(t) = y0 + sum_{k=0..18} D_k * (clamp(t, k, k+1) - k),  t = 19x.

Per [128, 2048] chunk the 19 slope-knots are split across engines (HW-measured
per-op costs for [128x2048]):
- 3 custom-DVE pair ops (2 knots each, 2311ns, self-accumulating, zero-seeded;
  custom DVE ops never hit perf modes so they run 1 elem/cycle)
- 9 stock DVE tensor_scalar clamp knots in fp16 (624ns -- 4x_2p perf mode)
- 4 ACT Relu knots (1903ns, telescoped coefficients, contiguous top block)
- PE accumulates ts/relu bases + the pair partial via diag fp16 matmuls into
  PSUM (512-col subtiles; redundant InstLdweights between same-weight subtile
  matmuls are deduped post-compile: ~53ns/load on HW, the split matmults are
  non-self-loading so one load serves the 4-subtile group)
- ACT also produces t16 = fp16(19x) (software-prefetched one chunk ahead) and
  the PSUM->SBUF copy, which carries the K = y0 - sum(D_k*k) constant in its
  free bias slot (so the pair chain seeds from an exact zero tile).

Engine busy per chunk ~ DVE 12.55us, ACT 11.4us, PE 12.7us vs DMA floor
~6.3us -- compute-bound at ~12.7us/chunk, ~347us/core measured (vs 391-399us
for the previous p=4/a=6/b=5 split without ldweights dedupe).

Sharding: pure data parallel -- batch b -> NeuronCore b (8 cores); the tiny
LUT/weight tensors ride along as per-core inputs. rel_err ~ 6.9e-3 (fp16
bases; gate is 2e-2).
"""

import sys

if "/opt/trn_rl_repo" not in sys.path:
    sys.path.insert(0, "/opt/trn_rl_repo")

import numpy as np

import concourse.bacc as bacc
import concourse.dve_ops as dve_ops_mod
import concourse.mybir as mybir
from concourse import bass_utils
from concourse.dve_ops import DveOp
from concourse.dve_spec import (
    C0,
    C1,
    C2,
    Latch,
    One,
    Spec,
    Src0,
    Src1,
    Zero,
    lower,
    maxx,
    minn,
    _has_src1,
)
from concourse.dve_uop import DveOpSpec
from concourse.tile import TileContext

N_BINS = 20
EPS = 1e-5
BATCH = 8
SPATIAL = (192, 192, 192)
N_ELEM = 192 * 192 * 192  # 7_077_888
P = 128
FREE = N_ELEM // P  # 55296
CHUNK = 2048
PSUM_TILE = 2048
PSUM_SUB = 512


# --------------------------------------------------------------------------
# Custom DVE pair op: out = s0*clamp(t-K,0,1) + s1*clamp(t-K,0,2) + acc
# --------------------------------------------------------------------------
def _pair_body():
    e = Src0 - C2
    r = maxx(e, Zero)
    c1 = minn(r, One)
    p1 = c1 * C0
    c2 = minn(r, One + One)
    a = Src1 + p1
    p2 = c2 * Latch(maxx(C1, C1))
    return a + p2


def _np_pair(in0, in1, s0, s1, imm2):
    e = in0.astype(np.float32) - np.float32(imm2)
    c1 = np.minimum(np.maximum(e, np.float32(0)), np.float32(1))
    c2 = np.minimum(np.maximum(e, np.float32(0)), np.float32(2))
    s0 = np.asarray(s0, dtype=np.float32)
    s1 = np.asarray(s1, dtype=np.float32)
    return (c1 * s0 + c2 * s1 + in1).astype(np.float32)


def _register(name: str, spec: Spec) -> DveOp:
    for op in dve_ops_mod.OPS:
        if op.name == name:
            return op
    row = dve_ops_mod._CUSTOM_DVE_ROW_BASE + len(dve_ops_mod.OPS)
    assert row < 0x20, "custom-DVE row overflow"
    sha = {}
    for ver in ("v3", "v4"):
        try:
            s = DveOpSpec(
                name=name,
                opcode=row,
                uops=lower(spec, ver=ver),
                rd1_en=_has_src1(spec),
            )
            sha[ver] = s.sha(ver)
        except Exception:
            pass
    op = DveOp(name, spec, subdim=False, uops_sha=sha)
    dve_ops_mod.OPS.append(op)
    dve_ops_mod.CUSTOM_DVE_SPECS[name] = spec
    dve_ops_mod._SUB_OPCODE_FOR_NAME[name] = row
    return op


AUGLUT_PAIR = _register(
    "AUGLUT_PAIR",
    Spec(
        body=_pair_body(),
        reference=lambda in0, in1, s0, s1, imm2: _np_pair(in0, in1, s0, s1, imm2),
    ),
)


# --------------------------------------------------------------------------
# Knot split
# --------------------------------------------------------------------------
def knot_split(p: int, a: int, b: int):
    """19 slope-knots -> (pair knots 0..2p-1, ts knots, relu top block).
    relu block is the contiguous top [19-b .. 18]; ts fills the middle."""
    assert 2 * p + a + b == 19, (p, a, b)
    pair_knots = list(range(0, 2 * p))
    relu_knots = list(range(19 - b, 19))
    ts_knots = [k for k in range(2 * p, 19 - b)]
    assert len(ts_knots) == a
    return pair_knots, ts_knots, relu_knots


# --------------------------------------------------------------------------
# Post-compile ldweights dedupe
# --------------------------------------------------------------------------
def _dedupe_ldweights(nc):
    """Drop PE InstLdweights that reload the exact weights already resident
    (same stationary AP as the previous ldweights, only matmuls in between).
    Only sync-free loads are removed; the split matmults are non-self-loading
    so the retained first load serves the whole subtile group. Verified
    numerically identical on HW."""
    for blk in nc.m.functions[0].blocks:
        insts = list(blk.instructions)
        out = []
        last_lw_key = None
        removed = 0
        for x in insts:
            if str(x.engine) == "EngineType.PE":
                nm = type(x).__name__
                if nm == "InstLdweights":
                    si = x.sync_info
                    clean = si is None or (
                        len(si.on_wait) == 0 and len(si.on_update) == 0
                    )
                    key = str(x.ins[0])
                    if clean and key == last_lw_key:
                        removed += 1
                        continue
                    last_lw_key = key
                elif nm not in ("InstMatmult", "InstMatmultMx"):
                    last_lw_key = None
            out.append(x)
        if removed:
            blk.instructions = out


# --------------------------------------------------------------------------
# Bass module
# --------------------------------------------------------------------------
def build_module(
    reps: int = 1,
    chunk: int = CHUNK,
    psum_tile: int = PSUM_TILE,
    p: int = 3,
    a: int = 9,
    b: int = 4,
    bufs: int = 4,
    basis_bufs: int = 8,
    psum_bufs: int = 2,
    psum_sub: int = PSUM_SUB,
    ts_first: bool = True,
    dedupe_lw: bool = True,
):
    nc = bacc.Bacc("TRN2", target_bir_lowering=False, debug=False, num_devices=BATCH)

    f32 = mybir.dt.float32
    f16 = mybir.dt.float16
    pair_knots, ts_knots, relu_knots = knot_split(p, a, b)
    assert p > 0
    n_mat = a + b + 1  # ts + relu + inject
    n_half = chunk // psum_tile
    assert n_half * psum_tile == chunk
    n_sub = psum_tile // psum_sub
    assert n_sub * psum_sub == psum_tile

    x_d = nc.dram_tensor("x", [P, FREE], f32, kind="ExternalInput")
    # lut[:, 0:2*p]: pair-op scalars (s0,s1 per pair)
    lut_d = nc.dram_tensor("lut", [P, N_BINS], f32, kind="ExternalInput")
    # wts: n_mat diag fp16 weight matrices [128, n_mat*128]
    wts_d = nc.dram_tensor("wts", [P, n_mat * P], f16, kind="ExternalInput")
    # ab[:, 0]: final bias K; ab[:, 1:1+b]: -k biases for the relu knots
    ab_d = nc.dram_tensor("ab", [P, 8], f32, kind="ExternalInput")
    o_d = nc.dram_tensor("o", [P, FREE], f32, kind="ExternalOutput")

    x_ap = x_d.ap()
    o_ap = o_d.ap()
    n_chunks = FREE // chunk
    assert n_chunks * chunk == FREE, (chunk, FREE)

    with TileContext(nc) as tc:
        with (
            tc.tile_pool(name="lutp", bufs=1) as lutp,
            tc.tile_pool(name="xp", bufs=bufs) as xp,
            tc.tile_pool(name="tp", bufs=bufs) as tp,
            tc.tile_pool(name="bp", bufs=basis_bufs) as bp,
            tc.tile_pool(name="op", bufs=max(2, bufs)) as op_,
            tc.tile_pool(name="pairp", bufs=max(2, bufs - 1)) as pairp,
            tc.tile_pool(name="psum", bufs=psum_bufs, space="PSUM") as pp,
        ):
            lut_t = lutp.tile([P, N_BINS], f32)
            nc.sync.dma_start(out=lut_t[:], in_=lut_d.ap()[:])
            wts_t = lutp.tile([P, n_mat * P], f16)
            nc.sync.dma_start(out=wts_t[:], in_=wts_d.ap()[:])
            ab_t = lutp.tile([P, 8], f32)
            nc.sync.dma_start(out=ab_t[:], in_=ab_d.ap()[:])
            # zero seed for the pair chain (fp16, exact)
            z16 = lutp.tile([P, chunk], f16)
            nc.vector.memset(z16[:], 0.0)

            def make_t16(j):
                sl = slice(j * chunk, (j + 1) * chunk)
                xt = xp.tile([P, chunk], f32, tag="x")
                nc.sync.dma_start(out=xt[:], in_=x_ap[:, sl])
                t16 = tp.tile([P, chunk], f16, tag="t16")
                nc.scalar.mul(out=t16[:], in_=xt[:], mul=19.0)
                return xt, t16

            def chunk_body(j, xt, t16, emit_next):
                nxt = emit_next()
                pss = []
                for _h in range(n_half):
                    ps_h = pp.tile([P, psum_tile], f32, tag="ps")
                    pss.append(ps_h)
                mi = 0

                def accumulate(basis_tile, mi):
                    w_sl = wts_t[:, mi * P : (mi + 1) * P]
                    for h in range(n_half):
                        for i in range(n_sub):
                            lo = i * psum_sub
                            nc.tensor.matmul(
                                pss[h][:, lo : lo + psum_sub],
                                w_sl,
                                basis_tile[
                                    :,
                                    h * psum_tile + lo : h * psum_tile + lo + psum_sub,
                                ],
                                start=(mi == 0),
                                stop=(mi == n_mat - 1),
                            )

                def emit_pairs():
                    acc = pairp.tile([P, chunk], f16, tag="acc")
                    accf = pairp.tile([P, chunk], f16, tag="accf")
                    prev = z16
                    for pr in range(p):
                        out_t = accf if pr == p - 1 else acc
                        nc.vector._custom_dve(
                            AUGLUT_PAIR,
                            out=out_t[:],
                            in0=t16[:],
                            in1=prev[:],
                            s0=lut_t[:, 2 * pr : 2 * pr + 1],
                            s1=lut_t[:, 2 * pr + 1 : 2 * pr + 2],
                            imm2=float(2 * pr),
                        )
                        prev = acc
                    return accf

                if not ts_first:
                    accf = emit_pairs()

                for k in ts_knots:
                    bt = bp.tile([P, chunk], f16, tag="basis")
                    nc.vector.tensor_scalar(
                        out=bt[:],
                        in0=t16[:],
                        scalar1=float(k),
                        scalar2=float(k + 1),
                        op0=mybir.AluOpType.max,
                        op1=mybir.AluOpType.min,
                    )
                    accumulate(bt, mi)
                    mi += 1

                for idx, k in enumerate(relu_knots):
                    rt = bp.tile([P, chunk], f16, tag="basis")
                    nc.scalar.activation(
                        out=rt[:],
                        in_=xt[:],
                        func=mybir.ActivationFunctionType.Relu,
                        bias=ab_t[:, 1 + idx : 2 + idx],
                        scale=19.0,
                    )
                    accumulate(rt, mi)
                    mi += 1

                if ts_first:
                    accf = emit_pairs()
                accumulate(accf, mi)
                mi += 1
                assert mi == n_mat, (mi, n_mat)

                # per-psum-tile copy with the K constant as free bias
                for h in range(n_half):
                    os_t = op_.tile([P, psum_tile], f32, tag="os")
                    nc.scalar.activation(
                        out=os_t[:],
                        in_=pss[h][:],
                        func=mybir.ActivationFunctionType.Identity,
                        bias=ab_t[:, 0:1],
                        scale=1.0,
                    )
                    nc.sync.dma_start(
                        out=o_ap[
                            :,
                            j * chunk + h * psum_tile : j * chunk + (h + 1) * psum_tile,
                        ],
                        in_=os_t[:],
                    )
                return nxt

            def body():
                cur = make_t16(0)
                for j in range(n_chunks):
                    emit_next = (
                        (lambda jj=j: make_t16(jj + 1))
                        if j + 1 < n_chunks
                        else (lambda: None)
                    )
                    cur = chunk_body(j, cur[0], cur[1], emit_next)

            if reps == 1:
                body()
            else:
                with tc.For_i(
                    0,
                    reps,
                    1,
                    hint_engines=(
                        mybir.EngineType.DVE,
                        mybir.EngineType.SP,
                        mybir.EngineType.Activation,
                        mybir.EngineType.PE,
                        mybir.EngineType.Pool,
                    ),
                ):
                    body()

    nc.finalize()
    if dedupe_lw:
        _dedupe_ldweights(nc)
    return nc


_MODULE_CACHE: dict[tuple, object] = {}


def _get_module(reps: int = 1, **cfg):
    key = (reps, tuple(sorted(cfg.items())))
    if key not in _MODULE_CACHE:
        _MODULE_CACHE[key] = build_module(reps, **cfg)
    return _MODULE_CACHE[key]


# --------------------------------------------------------------------------
# Host-side LUT prep
# --------------------------------------------------------------------------
def _make_luts(ran_y: np.ndarray, p: int, a: int, b: int):
    """ran_y [8,20] -> lut [8,128,20] f32 (pair scalars), wts [8,128,n_mat*128]
    f16 (diag weights: ts D_k, telescoped relu coefs, identity inject),
    ab [8,128,8] f32 (K bias + relu -k biases)."""
    y = ran_y.astype(np.float32)
    ymin = y.min(axis=1, keepdims=True)
    ymax = y.max(axis=1, keepdims=True)
    y = (y - ymin) / (ymax - ymin + np.float32(EPS))

    D = y[:, 1:] - y[:, :-1]  # [8, 19] slope deltas
    y0 = y[:, 0]

    pair_knots, ts_knots, relu_knots = knot_split(p, a, b)
    n_mat = a + b + 1

    # pair scalars: for pair j (knots 2j, 2j+1): s0 = D_2j - D_{2j+1}, s1 = D_{2j+1}
    lut = np.zeros((BATCH, N_BINS), np.float32)
    for j in range(p):
        lut[:, 2 * j] = D[:, 2 * j] - D[:, 2 * j + 1]
        lut[:, 2 * j + 1] = D[:, 2 * j + 1]
    lut_full = np.broadcast_to(lut[:, None, :], (BATCH, P, N_BINS)).copy()

    wts = np.zeros((BATCH, P, n_mat * P), np.float16)
    di = np.arange(P)
    K = y0.copy()
    mi = 0
    for k in ts_knots:
        wts[:, di, mi * P + di] = D[:, k][:, None].astype(np.float16)
        K = K - D[:, k] * np.float32(k)
        mi += 1
    # telescoped relu coefficients on the contiguous top block
    for idx, k in enumerate(relu_knots):
        coef = D[:, k] if idx == 0 else D[:, k] - D[:, k - 1]
        wts[:, di, mi * P + di] = coef[:, None].astype(np.float16)
        mi += 1
    wts[:, di, mi * P + di] = np.float16(1.0)  # inject identity
    mi += 1
    assert mi == n_mat

    ab = np.zeros((BATCH, P, 8), np.float32)
    ab[:, :, 0] = K[:, None]
    for idx, k in enumerate(relu_knots):
        ab[:, :, 1 + idx] = -np.float32(k)
    return lut_full, wts, ab


# --------------------------------------------------------------------------
# Entry point
# --------------------------------------------------------------------------
CFG = dict(p=3, a=9, b=4, bufs=4, basis_bufs=8, chunk=CHUNK, psum_tile=PSUM_TILE)


def kernel(x: np.ndarray, ran_y: np.ndarray, _reps: int = 1, **_cfg) -> np.ndarray:
    x = np.asarray(x, dtype=np.float32)
    ran_y = np.asarray(ran_y, dtype=np.float32)
    assert x.shape == (BATCH, *SPATIAL), x.shape
    assert ran_y.shape == (BATCH, N_BINS), ran_y.shape

    cfg = {**CFG, **_cfg}
    nc = _get_module(_reps, **cfg)
    lut, wts, ab = _make_luts(ran_y, cfg["p"], cfg["a"], cfg["b"])
    xr = np.ascontiguousarray(x.reshape(BATCH, P, FREE))
    in_maps = []
    for bi in range(BATCH):
        in_maps.append({"x": xr[bi], "lut": lut[bi], "wts": wts[bi], "ab": ab[bi]})

    res = bass_utils.run_bass_kernel_spmd(nc, in_maps, core_ids=list(range(BATCH)))
    out = np.stack([res.results[bi]["o"] for bi in range(BATCH)], axis=0)
    return out.reshape(BATCH, *SPATIAL)
